# revision 14
# baseline (speedup 1.0000x reference)
# Trainium2 Bass kernel for CustomFullyConnectedLayer:
#   y = x @ W.T,  W[(c+i)%N, c] += V[i, c] for i in diag_pos  (banded weight)
# Strategy: data-parallel over batch across 8 cores. Host pre-transposes the
# per-core x shard to feature-major xT_ext[k, b] = x[b, (k-29) % N] (29-row
# wraparound extension) and lays it out in DRAM exactly as the SBUF image
# [128, 25*1024], so every DMA piece is a large contiguous run per partition.
# For each of 24 output row-blocks rho (128 rows of y.T):
#   yT[128 rho + q, b] = sum_k w1[k, rho, q] * xT_ext[128 rho + k, b]
#                      + sum_j w2[j, rho, q] * xT_ext[128(rho+1) + j, b]
# i.e. one K=128 matmul plus one K=29 matmul accumulated in PSUM, per
# 512-wide batch half. Everything moves as bf16 (in and out), roughly
# halving HBM traffic vs an fp32-output version. A dense burst of dummy
# matmuls at kernel start opens the PE HAM clock gate (1.2 -> 2.4 GHz)
# before the first real matmul. Host un-swizzles yT back.
import os
import sys

import numpy as np

if "/opt/trn_rl_repo" not in sys.path:
    sys.path.insert(0, "/opt/trn_rl_repo")

import ml_dtypes

BATCH = 8192
N = 3072
NCORES = 8
BC = BATCH // NCORES          # 1024 batch rows per core
RB = N // 128                 # 24 output row-blocks
NS = RB + 1                   # 25 x feature slices of 128 (last one partial)
XROWS = NS * 128              # 3200 (rows 3101.. are zero padding)
EXT = 29                      # wraparound extension = max diag offset
NWARM = 135                   # PE clock-gate warm-up matmuls (also paces handoff)

_CACHE = {}
LAST_RESULTS = None


def _build_program():
    import concourse.mybir as mybir
    import concourse.tile as tile
    from concourse import bacc

    bf16 = mybir.dt.bfloat16
    f32 = mybir.dt.float32

    nc = bacc.Bacc("TRN2", target_bir_lowering=False, debug=False)
    # DRAM layouts mirror the SBUF images: partition-major, contiguous runs.
    xt = nc.dram_tensor("xt", [128, NS * BC], bf16, kind="ExternalInput")
    w1 = nc.dram_tensor("w1", [128, RB * 128], bf16, kind="ExternalInput")
    w2 = nc.dram_tensor("w2", [32, RB * 128], bf16, kind="ExternalInput")
    yt = nc.dram_tensor("yt", [128, RB * BC], bf16, kind="ExternalOutput")

    xtr = xt.rearrange("p (s b) -> p s b", b=BC)    # [128, 25, 1024]
    ytr = yt.rearrange("p (r b) -> p r b", b=BC)    # [128, 24, 1024]

    XCH = [2, 7, 13, 19, 24]      # x chunk boundaries (slices of 128 rows)
    SG = [4, 4, 4, 3, 3]          # y store groups (rho >= 18 streamed as halves)

    with tile.TileContext(nc) as tc:
        with (
            tc.tile_pool(name="consts", bufs=1) as consts,
            tc.tile_pool(name="xp", bufs=1) as xp,
            tc.tile_pool(name="yp", bufs=1) as yp,
            tc.tile_pool(name="pp", bufs=6, space="PSUM") as pp,
            tc.tile_pool(name="pw", bufs=1, space="PSUM") as pw,
        ):
            xall = xp.tile([128, NS, BC], bf16)
            yall = yp.tile([128, RB, BC], bf16)
            w1_sb = consts.tile([128, RB * 128], bf16)
            w2_sb = consts.tile([32, RB * 128], bf16)
            wsrc = consts.tile([128, 512], bf16)

            # PE warm-up: dense dummy matmuls while the DMAs fill, so the
            # HAM clock gate opens (1.2 -> 2.4 GHz) before the first real
            # matmul and stays open (steady-state PE gaps are < 1 us).
            nc.vector.memset(wsrc, 0.0)
            wps = pw.tile([128, 512], f32)
            for _ in range(NWARM):
                nc.tensor.matmul(
                    wps[:, 0:128], lhsT=wsrc[:, 0:128], rhs=wsrc[:, 0:128],
                    start=True, stop=True,
                )

            # first x chunk on the SP HWDGE ring; weights concurrently on the
            # ACT HWDGE ring so the first matmul isn't serialized behind both
            nc.sync.dma_start(out=xall[:, 0:XCH[0], :], in_=xtr[:, 0:XCH[0], :])
            nc.scalar.dma_start(out=w1_sb[:, 0:768], in_=w1[:, 0:768])
            nc.scalar.dma_start(out=w2_sb[:, 0:768], in_=w2[:, 0:768])
            nc.scalar.dma_start(out=w1_sb[:, 768:], in_=w1[:, 768:])
            nc.scalar.dma_start(out=w2_sb[:, 768:], in_=w2[:, 768:])
            # middle chunks ride the gpsimd/scalar rings (idle until stores
            # begin ~20us) so slow-HBM days aren't single-ring-limited
            xeng = [nc.sync, nc.gpsimd, nc.scalar, nc.sync]
            for eng, (c0, c1) in zip(xeng, zip(XCH[:-1], XCH[1:])):
                eng.dma_start(out=xall[:, c0:c1, :], in_=xtr[:, c0:c1, :])
            # partial last slice: only rows 3072..3103 hold data
            nc.sync.dma_start(
                out=xall[0:32, RB, :], in_=xtr[0:32, RB, :]
            )

            g_start, g_i, done = 0, 0, 0
            for rho in range(RB):
                lt1 = w1_sb[:, rho * 128:(rho + 1) * 128]
                lt2 = w2_sb[0:EXT, rho * 128:(rho + 1) * 128]
                ps0 = pp.tile([128, 512], f32, tag="ps")
                ps1 = pp.tile([128, 512], f32, tag="ps")
                nc.tensor.matmul(
                    ps0, lhsT=lt1, rhs=xall[:, rho, 0:512],
                    start=True, stop=False,
                )
                nc.tensor.matmul(
                    ps1, lhsT=lt1, rhs=xall[:, rho, 512:1024],
                    start=True, stop=False,
                )
                nc.tensor.matmul(
                    ps0, lhsT=lt2, rhs=xall[0:EXT, rho + 1, 0:512],
                    start=False, stop=True,
                )
                nc.tensor.matmul(
                    ps1, lhsT=lt2, rhs=xall[0:EXT, rho + 1, 512:1024],
                    start=False, stop=True,
                )
                if rho >= 18:
                    # tail blocks: store each half right after its copy, on
                    # parallel rings, so stores trail copies by <2 us
                    nc.vector.tensor_copy(out=yall[:, rho, 0:512], in_=ps0)
                    nc.gpsimd.dma_start(
                        out=ytr[:, rho, 0:512], in_=yall[:, rho, 0:512]
                    )
                    nc.scalar.copy(out=yall[:, rho, 512:1024], in_=ps1)
                    nc.scalar.dma_start(
                        out=ytr[:, rho, 512:1024], in_=yall[:, rho, 512:1024]
                    )
                    continue
                nc.vector.tensor_copy(out=yall[:, rho, 0:512], in_=ps0)
                nc.scalar.copy(out=yall[:, rho, 512:1024], in_=ps1)
                done += 1
                if done == SG[g_i]:
                    g_end = g_start + SG[g_i]
                    eng = nc.gpsimd if g_i % 2 == 0 else nc.scalar
                    eng.dma_start(
                        out=ytr[:, g_start:g_end, :],
                        in_=yall[:, g_start:g_end, :],
                    )
                    g_start, g_i, done = g_end, g_i + 1, 0

    nc.compile()
    return nc


def _host_prep(x, V, diag_pos):
    bf16 = ml_dtypes.bfloat16
    x = np.asarray(x, dtype=np.float32)
    V = np.asarray(V, dtype=np.float32)
    diag = np.asarray(diag_pos).astype(np.int64) % N
    if diag.size and int(diag.max()) > EXT:
        raise ValueError(
            f"band kernel supports diag offsets <= {EXT}, got {int(diag.max())}"
        )

    # w1[k, rho, q] = V[i, c], i = q + EXT - k,  c = (128 rho + k - EXT) % N
    # w2[j, rho, q] = V[i, c], i = q - (128 - EXT) - j, c = (128 rho + 128 - EXT + j) % N
    w1 = np.zeros((128, RB, 128), np.float32)
    w2 = np.zeros((32, RB, 128), np.float32)
    rho = np.arange(RB)[:, None]
    q = np.arange(128)[None, :]
    for i in diag:
        i = int(i)
        k = q + EXT - i                              # [1, 128], all >= 0
        validA = k < 128
        cA = (128 * rho + k - EXT) % N               # [RB, 128]
        kk = np.broadcast_to(k, (RB, 128))
        rr = np.broadcast_to(rho, (RB, 128))
        qq = np.broadcast_to(q, (RB, 128))
        m = np.broadcast_to(validA, (RB, 128))
        w1[kk[m], rr[m], qq[m]] += V[i, cA[m]]
        j = q - (128 - EXT) - i
        validB = (j >= 0) & (j < EXT)
        cB = (128 * rho + 128 - EXT + j) % N
        jj = np.broadcast_to(j, (RB, 128))
        m = np.broadcast_to(validB, (RB, 128))
        w2[jj[m], rr[m], qq[m]] += V[i, cB[m]]

    # [k, rho, q] -> [k, rho*128 + q] contiguous per partition
    w1 = np.ascontiguousarray(w1.reshape(128, RB * 128).astype(bf16))
    w2 = np.ascontiguousarray(w2.reshape(32, RB * 128).astype(bf16))

    # xT_ext[k', b] = x[b, (k'-EXT) % N]; swizzled per-core later
    xb = x.astype(bf16)                              # contiguous cast first
    xt_all = np.zeros((XROWS, BATCH), bf16)
    xt_all[EXT:EXT + N] = xb.T
    xt_all[0:EXT] = xb.T[N - EXT:]
    # swizzle to the SBUF image: xs[s, p, b] = xT_ext[128 s + p, b]
    xs_all = xt_all.reshape(NS, 128, BATCH)
    return xs_all, w1, w2


def kernel(x, V, diag_pos):
    global LAST_RESULTS
    from concourse.bass_utils import run_bass_kernel_spmd

    if "prog" not in _CACHE:
        _CACHE["prog"] = _build_program()
    nc = _CACHE["prog"]

    xs_all, w1, w2 = _host_prep(x, V, diag_pos)
    in_maps = []
    for k in range(NCORES):
        sl = xs_all[:, :, k * BC:(k + 1) * BC]       # [25, 128, 1024]
        xt_core = np.ascontiguousarray(
            sl.transpose(1, 0, 2)                    # [128, 25, 1024]
        ).reshape(128, NS * BC)
        in_maps.append({"xt": xt_core, "w1": w1, "w2": w2})
    res = run_bass_kernel_spmd(nc, in_maps, core_ids=list(range(NCORES)))
    LAST_RESULTS = res
    parts = []
    for r in res.results:
        yt = r["yt"].reshape(128, RB, BC)            # [q, rho, b]
        parts.append(yt.transpose(2, 1, 0).reshape(BC, N))
    out = np.concatenate(parts, axis=0)
    return np.ascontiguousarray(out.astype(np.float32))


# revision 15
# speedup vs baseline: 1.4591x; 1.4591x over previous
# Trainium2 Bass kernel for CustomFullyConnectedLayer:
#   y = x @ W.T,  W[(c+i)%N, c] += V[i, c] for i in diag_pos  (banded weight)
# Strategy: data-parallel over batch across 8 cores. Host pre-transposes the
# per-core x shard to feature-major xT_ext[k, b] = x[b, (k-29) % N] (29-row
# wraparound extension) and lays it out in DRAM exactly as the SBUF image
# [128, 25*1024], so every DMA piece is a large contiguous run per partition.
# For each of 24 output row-blocks rho (128 rows of y.T):
#   yT[128 rho + q, b] = sum_k w1[k, rho, q] * xT_ext[128 rho + k, b]
#                      + sum_j w2[j, rho, q] * xT_ext[128(rho+1) + j, b]
# i.e. one K=128 matmul plus one K=29 matmul accumulated in PSUM, per
# 512-wide batch half. Everything moves as bf16 (in and out), roughly
# halving HBM traffic vs an fp32-output version. A dense burst of dummy
# matmuls at kernel start opens the PE HAM clock gate (1.2 -> 2.4 GHz)
# before the first real matmul. Host un-swizzles yT back.
import os
import sys

import numpy as np

if "/opt/trn_rl_repo" not in sys.path:
    sys.path.insert(0, "/opt/trn_rl_repo")

import ml_dtypes

BATCH = 8192
N = 3072
NCORES = 8
BC = BATCH // NCORES          # 1024 batch rows per core
RB = N // 128                 # 24 output row-blocks
NS = RB + 1                   # 25 x feature slices of 128 (last one partial)
XROWS = NS * 128              # 3200 (rows 3101.. are zero padding)
EXT = 29                      # wraparound extension = max diag offset
NWARM = 135                   # PE clock-gate warm-up matmuls (also paces handoff)

_CACHE = {}
LAST_RESULTS = None


def _build_program():
    import concourse.mybir as mybir
    import concourse.tile as tile
    from concourse import bacc

    bf16 = mybir.dt.bfloat16
    f32 = mybir.dt.float32

    nc = bacc.Bacc("TRN2", target_bir_lowering=False, debug=False)
    # DRAM layouts mirror the SBUF images: partition-major, contiguous runs.
    xt = nc.dram_tensor("xt", [128, NS * BC], bf16, kind="ExternalInput")
    w1 = nc.dram_tensor("w1", [128, RB * 128], bf16, kind="ExternalInput")
    w2 = nc.dram_tensor("w2", [32, RB * 128], bf16, kind="ExternalInput")
    yt = nc.dram_tensor("yt", [128, RB * BC], bf16, kind="ExternalOutput")

    xtr = xt.rearrange("p (s b) -> p s b", b=BC)    # [128, 25, 1024]
    ytr = yt.rearrange("p (r b) -> p r b", b=BC)    # [128, 24, 1024]

    XCH = [2, 7, 13, 19, 24]      # x chunk boundaries (slices of 128 rows)
    SG = [4, 4, 4, 3, 3]          # y store groups (rho >= 18 streamed as halves)

    with tile.TileContext(nc) as tc:
        with (
            tc.tile_pool(name="consts", bufs=1) as consts,
            tc.tile_pool(name="xp", bufs=1) as xp,
            tc.tile_pool(name="yp", bufs=1) as yp,
            tc.tile_pool(name="pp", bufs=6, space="PSUM") as pp,
            tc.tile_pool(name="pw", bufs=1, space="PSUM") as pw,
        ):
            xall = xp.tile([128, NS, BC], bf16)
            yall = yp.tile([128, RB, BC], bf16)
            w1_sb = consts.tile([128, RB * 128], bf16)
            w2_sb = consts.tile([32, RB * 128], bf16)
            wsrc = consts.tile([128, 512], bf16)

            # PE warm-up: dense dummy matmuls while the DMAs fill, so the
            # HAM clock gate opens (1.2 -> 2.4 GHz) before the first real
            # matmul and stays open (steady-state PE gaps are < 1 us).
            nc.vector.memset(wsrc, 0.0)
            wps = pw.tile([128, 512], f32)
            for _ in range(NWARM):
                nc.tensor.matmul(
                    wps[:, 0:128], lhsT=wsrc[:, 0:128], rhs=wsrc[:, 0:128],
                    start=True, stop=True,
                )

            # first x chunk on the SP HWDGE ring; weights concurrently on the
            # ACT HWDGE ring so the first matmul isn't serialized behind both
            nc.sync.dma_start(out=xall[:, 0:XCH[0], :], in_=xtr[:, 0:XCH[0], :])
            nc.scalar.dma_start(out=w1_sb[:, 0:768], in_=w1[:, 0:768])
            nc.scalar.dma_start(out=w2_sb[:, 0:768], in_=w2[:, 0:768])
            nc.scalar.dma_start(out=w1_sb[:, 768:], in_=w1[:, 768:])
            nc.scalar.dma_start(out=w2_sb[:, 768:], in_=w2[:, 768:])
            for c0, c1 in zip(XCH[:-1], XCH[1:]):
                nc.sync.dma_start(out=xall[:, c0:c1, :], in_=xtr[:, c0:c1, :])
            # partial last slice: only rows 3072..3103 hold data
            nc.sync.dma_start(
                out=xall[0:32, RB, :], in_=xtr[0:32, RB, :]
            )

            g_start, g_i, done = 0, 0, 0
            for rho in range(RB):
                lt1 = w1_sb[:, rho * 128:(rho + 1) * 128]
                lt2 = w2_sb[0:EXT, rho * 128:(rho + 1) * 128]
                ps0 = pp.tile([128, 512], f32, tag="ps")
                ps1 = pp.tile([128, 512], f32, tag="ps")
                nc.tensor.matmul(
                    ps0, lhsT=lt1, rhs=xall[:, rho, 0:512],
                    start=True, stop=False,
                )
                nc.tensor.matmul(
                    ps1, lhsT=lt1, rhs=xall[:, rho, 512:1024],
                    start=True, stop=False,
                )
                nc.tensor.matmul(
                    ps0, lhsT=lt2, rhs=xall[0:EXT, rho + 1, 0:512],
                    start=False, stop=True,
                )
                nc.tensor.matmul(
                    ps1, lhsT=lt2, rhs=xall[0:EXT, rho + 1, 512:1024],
                    start=False, stop=True,
                )
                if rho >= 18:
                    # tail blocks: store each half right after its copy, on
                    # parallel rings, so stores trail copies by <2 us
                    nc.vector.tensor_copy(out=yall[:, rho, 0:512], in_=ps0)
                    nc.gpsimd.dma_start(
                        out=ytr[:, rho, 0:512], in_=yall[:, rho, 0:512]
                    )
                    nc.scalar.copy(out=yall[:, rho, 512:1024], in_=ps1)
                    nc.scalar.dma_start(
                        out=ytr[:, rho, 512:1024], in_=yall[:, rho, 512:1024]
                    )
                    continue
                nc.vector.tensor_copy(out=yall[:, rho, 0:512], in_=ps0)
                nc.scalar.copy(out=yall[:, rho, 512:1024], in_=ps1)
                done += 1
                if done == SG[g_i]:
                    g_end = g_start + SG[g_i]
                    eng = nc.gpsimd if g_i % 2 == 0 else nc.scalar
                    eng.dma_start(
                        out=ytr[:, g_start:g_end, :],
                        in_=yall[:, g_start:g_end, :],
                    )
                    g_start, g_i, done = g_end, g_i + 1, 0

    nc.compile()
    return nc


def _host_prep(x, V, diag_pos):
    bf16 = ml_dtypes.bfloat16
    x = np.asarray(x, dtype=np.float32)
    V = np.asarray(V, dtype=np.float32)
    diag = np.asarray(diag_pos).astype(np.int64) % N
    if diag.size and int(diag.max()) > EXT:
        raise ValueError(
            f"band kernel supports diag offsets <= {EXT}, got {int(diag.max())}"
        )

    # w1[k, rho, q] = V[i, c], i = q + EXT - k,  c = (128 rho + k - EXT) % N
    # w2[j, rho, q] = V[i, c], i = q - (128 - EXT) - j, c = (128 rho + 128 - EXT + j) % N
    w1 = np.zeros((128, RB, 128), np.float32)
    w2 = np.zeros((32, RB, 128), np.float32)
    rho = np.arange(RB)[:, None]
    q = np.arange(128)[None, :]
    for i in diag:
        i = int(i)
        k = q + EXT - i                              # [1, 128], all >= 0
        validA = k < 128
        cA = (128 * rho + k - EXT) % N               # [RB, 128]
        kk = np.broadcast_to(k, (RB, 128))
        rr = np.broadcast_to(rho, (RB, 128))
        qq = np.broadcast_to(q, (RB, 128))
        m = np.broadcast_to(validA, (RB, 128))
        w1[kk[m], rr[m], qq[m]] += V[i, cA[m]]
        j = q - (128 - EXT) - i
        validB = (j >= 0) & (j < EXT)
        cB = (128 * rho + 128 - EXT + j) % N
        jj = np.broadcast_to(j, (RB, 128))
        m = np.broadcast_to(validB, (RB, 128))
        w2[jj[m], rr[m], qq[m]] += V[i, cB[m]]

    # [k, rho, q] -> [k, rho*128 + q] contiguous per partition
    w1 = np.ascontiguousarray(w1.reshape(128, RB * 128).astype(bf16))
    w2 = np.ascontiguousarray(w2.reshape(32, RB * 128).astype(bf16))

    # xT_ext[k', b] = x[b, (k'-EXT) % N]; swizzled per-core later
    xb = x.astype(bf16)                              # contiguous cast first
    xt_all = np.zeros((XROWS, BATCH), bf16)
    xt_all[EXT:EXT + N] = xb.T
    xt_all[0:EXT] = xb.T[N - EXT:]
    # swizzle to the SBUF image: xs[s, p, b] = xT_ext[128 s + p, b]
    xs_all = xt_all.reshape(NS, 128, BATCH)
    return xs_all, w1, w2


def kernel(x, V, diag_pos):
    global LAST_RESULTS
    from concourse.bass_utils import run_bass_kernel_spmd

    if "prog" not in _CACHE:
        _CACHE["prog"] = _build_program()
    nc = _CACHE["prog"]

    xs_all, w1, w2 = _host_prep(x, V, diag_pos)
    in_maps = []
    for k in range(NCORES):
        sl = xs_all[:, :, k * BC:(k + 1) * BC]       # [25, 128, 1024]
        xt_core = np.ascontiguousarray(
            sl.transpose(1, 0, 2)                    # [128, 25, 1024]
        ).reshape(128, NS * BC)
        in_maps.append({"xt": xt_core, "w1": w1, "w2": w2})
    res = run_bass_kernel_spmd(nc, in_maps, core_ids=list(range(NCORES)))
    LAST_RESULTS = res
    parts = []
    for r in res.results:
        yt = r["yt"].reshape(128, RB, BC)            # [q, rho, b]
        parts.append(yt.transpose(2, 1, 0).reshape(BC, N))
    out = np.concatenate(parts, axis=0)
    return np.ascontiguousarray(out.astype(np.float32))
